# revision 21
# baseline (speedup 1.0000x reference)
"""Trainium2 Bass kernel for nn_MultiHeadAttention_90005334655147.

Math (faithful to the reference):
    qh = (q @ Wq + bq)  -> [B,S,H,DK] -> heads
    kh = (k @ Wk + bk)
    vh = (v @ Wk + bk)          # reference really uses Wk/bk for v
    S_ = (qh @ kh^T) / 8        # per head
    P  = softmax(S_) (mask is all-False in the harness inputs; a masked
                      fallback variant is compiled lazily if needed)
    out_h = P @ vh
    out = concat_h(out_h) @ Wfc + bfc + q   (residual)
    out = LayerNorm(out) * gamma + beta
    returns (out, P)

Sharding: data-parallel over (batch, query-block): core c handles batch
c//4 and query rows [(c%4)*512, (c%4)*512+512). Each core computes kh/vh
for its full batch (duplicated, cheap) -> zero collectives.

Device-side layout strategy (per core):
  - host pre-transposes/casts activations: qT/kT/vT = x.T in bf16 so the
    d-contraction sits on partitions; residual q stays natural fp32.
  - qhT/khT computed as [hdk, rows] (transposed), vh as [rows, hdv]:
    all via natural-layout matmuls.
  - scores are computed TWICE (cheap on PE): once as [q,k] (for the attn
    output + softmax sums) and once as [k,q] (feeds P@V directly).  This
    avoids any on-chip transpose of the big P matrix, whose cost would
    exceed the extra exp pass.
  - exp on ACT with fused accumulate (row sums); normalization on DVE.
  - LayerNorm rstd via exp(-0.5*ln(var+eps)) so ACT stays on the
    natural_log_exp table set (no table switches).
"""

import os
import sys

for _p in ("/opt/trn_rl_repo", "/root/.axon_site/_ro/trn_rl_repo"):
    if os.path.isdir(_p) and _p not in sys.path:
        sys.path.insert(0, _p)

import ml_dtypes
import numpy as np

import concourse.bacc as bacc
import concourse.mybir as mybir
import concourse.tile as tile
from concourse.bass_interp import get_hw_module
from concourse.bass_utils import run_bass_kernel_spmd

B, S, D = 2, 2048, 512
H, DK, DV = 8, 64, 64
HDK = H * DK  # 512
HDV = H * DV  # 512
INV_SCALE = 0.125  # attn / 8.0
LN_EPS = 1e-5

NCORES = 8
CORES_PER_B = NCORES // B  # 4
QPC = S // CORES_PER_B  # 512 query rows per core

F32 = mybir.dt.float32
BF16 = mybir.dt.bfloat16
BF = ml_dtypes.bfloat16

TRACE = bool(int(os.environ.get("KERNEL_TRACE", "0")))
LAST_EXEC_NS = None

if TRACE:
    # Register the axon NTFF profile hook if the image's antenv lacks it.
    try:
        import antenv.axon_hooks as _ah
        from trn_agent_boot.trn_boot import _ntff_profile_via_ctypes

        if _ah.get_axon_ntff_profile_hook() is None:
            _h = _ntff_profile_via_ctypes("/opt/axon/libaxon_pjrt.so")
            if _h is not None:
                _ah.set_axon_ntff_profile_hook(_h)
    except Exception:
        pass

_cache = {}

ADD = mybir.AluOpType.add
SUB = mybir.AluOpType.subtract
MULT = mybir.AluOpType.mult
EXP = mybir.ActivationFunctionType.Exp
LN_ = mybir.ActivationFunctionType.Ln


def _emit(nc, tc, io, use_mask, ctx):
    NQC = QPC // 128  # 4 query-row chunks
    NKC = S // 128  # 16 key-row chunks
    NDC = D // 128  # 4 d chunks
    NMC = HDK // 128  # 4 output-col chunks for projections
    SCT = 1024  # scores psum tile free size (2 banks)
    NSC = S // SCT  # 2 per (h,qc)

    singles = ctx.enter_context(tc.tile_pool(name="singles", bufs=1))
    p32p = ctx.enter_context(tc.tile_pool(name="p32p", bufs=2))
    ptp = ctx.enter_context(tc.tile_pool(name="ptp", bufs=4))
    smallp = ctx.enter_context(tc.tile_pool(name="smallp", bufs=6))
    outp = ctx.enter_context(tc.tile_pool(name="outp", bufs=2))
    ps_sc = ctx.enter_context(tc.tile_pool(name="ps_sc", bufs=2, space="PSUM"))
    ps_sct = ctx.enter_context(tc.tile_pool(name="ps_sct", bufs=2, space="PSUM"))
    ps_pv = ctx.enter_context(tc.tile_pool(name="ps_pv", bufs=2, space="PSUM"))
    if use_mask:
        maskqp = ctx.enter_context(tc.tile_pool(name="maskqp", bufs=2))
        masktp = ctx.enter_context(tc.tile_pool(name="masktp", bufs=2))

    # ---- stage 0: preload constants / inputs ----
    qT_sb = singles.tile([128, NDC, QPC], BF16, tag="qT_sb")
    nc.sync.dma_start(out=qT_sb, in_=io["qT"].rearrange("(c p) q -> p c q", p=128))
    kT_sb = singles.tile([128, NDC, S], BF16, tag="kT_sb")
    nc.sync.dma_start(out=kT_sb, in_=io["kT"].rearrange("(c p) q -> p c q", p=128))
    vT_sb = singles.tile([128, NDC, S], BF16, tag="vT_sb")
    nc.sync.dma_start(out=vT_sb, in_=io["vT"].rearrange("(c p) q -> p c q", p=128))
    qres_sb = singles.tile([128, NQC, D], F32, tag="qres_sb")
    nc.sync.dma_start(out=qres_sb, in_=io["qres"].rearrange("(c p) d -> p c d", p=128))

    wq_sb = singles.tile([128, NDC, HDK], BF16, tag="wq_sb")
    nc.sync.dma_start(out=wq_sb, in_=io["wq"].rearrange("(c p) m -> p c m", p=128))
    wk_sb = singles.tile([128, NDC, HDK], BF16, tag="wk_sb")
    nc.sync.dma_start(out=wk_sb, in_=io["wk"].rearrange("(c p) m -> p c m", p=128))
    wfc_sb = singles.tile([64, H, D], BF16, tag="wfc_sb")
    nc.sync.dma_start(out=wfc_sb, in_=io["wfc"].rearrange("(h p) n -> p h n", p=64))

    bq_sb = singles.tile([128, NMC], F32, tag="bq_sb")
    nc.sync.dma_start(out=bq_sb, in_=io["bq"].rearrange("(c p) -> p c", p=128))
    bk_sb = singles.tile([128, NMC], F32, tag="bk_sb")
    nc.sync.dma_start(out=bk_sb, in_=io["bk"].rearrange("(c p) -> p c", p=128))

    def bcast_row(name):
        t = singles.tile([128, D], F32, tag=name + "B")
        src = io[name].rearrange("(o n) -> o n", o=1).partition_broadcast(128)
        nc.sync.dma_start(out=t, in_=src)
        return t

    bkB = bcast_row("bk")
    bfcB = bcast_row("bfc")
    gammaB = bcast_row("gamma")
    betaB = bcast_row("beta")

    # vh with a ones column appended (row sums of exp come out of the PV
    # matmul for free, in [1, q] orientation)
    vh_sb = singles.tile([128, NKC, H, DV + 1], BF16, tag="vh_sb")
    nc.vector.memset(vh_sb[:, :, :, DV : DV + 1], 1.0)

    # ---- stage 1: projections ----
    qhT_sb = singles.tile([128, NMC, QPC], BF16, tag="qhT_sb")
    for m in range(NMC):
        ps = ps_sct.tile([128, QPC], F32, tag="sct")
        for c in range(NDC):
            nc.tensor.matmul(
                ps,
                lhsT=wq_sb[:, c, m * 128 : (m + 1) * 128],
                rhs=qT_sb[:, c, :],
                start=(c == 0),
                stop=(c == NDC - 1),
            )
        nc.vector.tensor_scalar_add(
            out=qhT_sb[:, m, :], in0=ps, scalar1=bq_sb[:, m : m + 1]
        )

    khT_sb = singles.tile([128, NMC, S], BF16, tag="khT_sb")
    for m in range(NMC):
        for n in range(NSC):
            ps = ps_sc.tile([128, SCT], F32, tag="sc")
            for half in range(SCT // 512):
                for c in range(NDC):
                    nc.tensor.matmul(
                        ps[:, half * 512 : (half + 1) * 512],
                        lhsT=wk_sb[:, c, m * 128 : (m + 1) * 128],
                        rhs=kT_sb[
                            :, c, n * SCT + half * 512 : n * SCT + (half + 1) * 512
                        ],
                        start=(c == 0),
                        stop=(c == NDC - 1),
                    )
            nc.vector.tensor_scalar_add(
                out=khT_sb[:, m, n * SCT : (n + 1) * SCT],
                in0=ps,
                scalar1=bk_sb[:, m : m + 1],
            )

    for kc in range(NKC):
        ps = ps_sct.tile([128, HDV], F32, tag="sct")
        for c in range(NDC):
            nc.tensor.matmul(
                ps,
                lhsT=vT_sb[:, c, kc * 128 : (kc + 1) * 128],
                rhs=wk_sb[:, c, :],
                start=(c == 0),
                stop=(c == NDC - 1),
            )
        nc.vector.tensor_tensor(
            out=vh_sb[:, kc, :, 0:DV],
            in0=ps.rearrange("p (h d) -> p h d", h=H),
            in1=bkB.rearrange("p (h d) -> p h d", h=H),
            op=ADD,
        )

    # ---- stage 2: attention per head ----
    outTn_sb = singles.tile([64, H, QPC], BF16, tag="outTn_sb")
    ones64 = singles.tile([65, 64], F32, tag="ones64")
    nc.vector.memset(ones64, 1.0)

    # Heads are processed in pairs: the even head's dk rows live on
    # partitions 0-63, the odd head's on 64-127, so their K=64 matmuls
    # issue to distinct PE row-groups (tile_position auto-derived from
    # base_partition) and run concurrently in the array.
    for pr in range(H // 2):
        heads = (2 * pr, 2 * pr + 1)
        pvs = {h: ps_pv.tile([DV + 1, QPC], F32, tag="pv", name=f"pv{h}") for h in heads}
        for kc in range(NKC):
            # scores^T [k,q] for both heads (packed pair)
            pss = {}
            for h in heads:
                hp = (h % 2) * 64
                ps = ps_sct.tile([128, QPC], F32, tag="sct", name=f"sct{h}_{kc}")
                nc.tensor.matmul(
                    ps,
                    lhsT=khT_sb[hp : hp + 64, pr, kc * 128 : (kc + 1) * 128],
                    rhs=qhT_sb[hp : hp + 64, pr, :],
                    start=True,
                    stop=True,
                )
                pss[h] = ps
            if use_mask:
                mt = masktp.tile([128, QPC], F32, tag="mt")
                nc.sync.dma_start(
                    out=mt, in_=io["maskbT"][kc * 128 : (kc + 1) * 128, :]
                )
                for h in heads:
                    nc.vector.tensor_tensor(out=pss[h], in0=pss[h], in1=mt, op=ADD)
            pts = {}
            for h in heads:
                pt = ptp.tile([128, QPC], BF16, tag="pt", name=f"pt{h}_{kc}")
                nc.scalar.activation(out=pt, in_=pss[h], func=EXP, scale=INV_SCALE)
                pts[h] = pt
            for h in heads:
                nc.tensor.matmul(
                    pvs[h],
                    lhsT=vh_sb[:, kc, h, :],
                    rhs=pts[h],
                    start=(kc == 0),
                    stop=(kc == NKC - 1),
                )

            # every 4th k-chunk, interleave one [q,k] scores burst so the
            # PE/ACT streams stay dense
            if kc % 4 == 3:
                qc = kc // 4
                p32s = {h: p32p.tile([128, S], F32, tag="p32", name=f"p32_{h}") for h in heads}
                sums = {
                    h: smallp.tile([128, NSC + 2], F32, tag="sums2", name=f"sums{h}") for h in heads
                }
                for n in range(NSC):
                    pp = {h: ps_sc.tile([128, SCT], F32, tag="sc", name=f"sc{h}_{n}") for h in heads}
                    for half in range(SCT // 512):
                        for h in heads:
                            hp = (h % 2) * 64
                            o = n * SCT + half * 512
                            nc.tensor.matmul(
                                pp[h][:, half * 512 : (half + 1) * 512],
                                lhsT=qhT_sb[
                                    hp : hp + 64, pr, qc * 128 : (qc + 1) * 128
                                ],
                                rhs=khT_sb[hp : hp + 64, pr, o : o + 512],
                                start=True,
                                stop=True,
                            )
                    if use_mask:
                        mq = maskqp.tile([128, SCT], F32, tag="mq")
                        nc.sync.dma_start(
                            out=mq,
                            in_=io["maskb"][
                                qc * 128 : (qc + 1) * 128, n * SCT : (n + 1) * SCT
                            ],
                        )
                        for h in heads:
                            nc.vector.tensor_tensor(
                                out=pp[h], in0=pp[h], in1=mq, op=ADD
                            )
                    for h in heads:
                        nc.scalar.activation(
                            out=p32s[h][:, n * SCT : (n + 1) * SCT],
                            in_=pp[h],
                            func=EXP,
                            scale=INV_SCALE,
                            accum_out=sums[h][:, n : n + 1],
                        )
                for h in heads:
                    s2 = sums[h]
                    nc.vector.tensor_tensor(
                        out=s2[:, NSC : NSC + 1],
                        in0=s2[:, 0:1],
                        in1=s2[:, 1:2],
                        op=ADD,
                    )
                    nc.vector.reciprocal(
                        out=s2[:, NSC + 1 : NSC + 2], in_=s2[:, NSC : NSC + 1]
                    )
                    nc.vector.tensor_scalar_mul(
                        out=p32s[h], in0=p32s[h], scalar1=s2[:, NSC + 1 : NSC + 2]
                    )
                    nc.sync.dma_start(
                        out=io["attn_o"][h, qc * 128 : (qc + 1) * 128, :],
                        in_=p32s[h],
                    )

        # normalize PV output rows by the (transposed) softmax sums:
        # reciprocal of the ones-column row, broadcast across partitions
        # via a K=1 outer-product matmul (all on-chip, partition-aligned)
        for h in heads:
            rt = smallp.tile([65, QPC], F32, tag="rt")
            nc.vector.reciprocal(out=rt[64:65, :], in_=pvs[h][DV : DV + 1, :])
            rtp = ps_sc.tile([64, QPC], F32, tag="sc")
            nc.tensor.matmul(
                rtp, lhsT=ones64[64:65, :], rhs=rt[64:65, :], start=True, stop=True
            )
            rtb = smallp.tile([64, QPC], F32, tag="rtb")
            nc.vector.tensor_copy(out=rtb, in_=rtp)
            nc.vector.tensor_tensor(
                out=outTn_sb[:, h, :], in0=pvs[h][0:DV, :], in1=rtb, op=MULT
            )

    # ---- stage 3: fc + residual + layernorm ----
    # All Ln/Exp calls batched at the end so the ACT table set is switched
    # at most twice instead of per-qc.
    eps_sb = singles.tile([128, 1], F32, tag="eps_sb")
    nc.vector.memset(eps_sb, LN_EPS)
    mvs = singles.tile([128, NQC, 2], F32, tag="mvs")
    lns = singles.tile([128, NQC], F32, tag="lns")
    rstds = singles.tile([128, NQC], F32, tag="rstds")
    xs = []
    for qc in range(NQC):
        fc = ps_sc.tile([128, D], F32, tag="sc")
        for h in range(H):
            nc.tensor.matmul(
                fc,
                lhsT=outTn_sb[:, h, qc * 128 : (qc + 1) * 128],
                rhs=wfc_sb[:, h, :],
                start=(h == 0),
                stop=(h == H - 1),
            )
        x = outp.tile([128, D], F32, tag="x", bufs=NQC)
        nc.vector.tensor_tensor(out=x, in0=fc, in1=qres_sb[:, qc, :], op=ADD)
        nc.vector.tensor_tensor(out=x, in0=x, in1=bfcB, op=ADD)
        st = smallp.tile([128, 6], F32, tag="st")
        nc.vector.bn_stats(out=st, in_=x)
        nc.vector.bn_aggr(out=mvs[:, qc, :], in_=st)
        xs.append(x)
    nc.scalar.activation(out=lns, in_=mvs[:, :, 1], func=LN_, bias=eps_sb, scale=1.0)
    nc.scalar.activation(out=rstds, in_=lns, func=EXP, scale=-0.5)
    for qc in range(NQC):
        y = outp.tile([128, D], F32, tag="y")
        nc.vector.tensor_scalar(
            out=y,
            in0=xs[qc],
            scalar1=mvs[:, qc, 0:1],
            scalar2=rstds[:, qc : qc + 1],
            op0=SUB,
            op1=MULT,
        )
        nc.vector.tensor_tensor(out=y, in0=y, in1=gammaB, op=MULT)
        nc.vector.tensor_tensor(out=y, in0=y, in1=betaB, op=ADD)
        nc.sync.dma_start(out=io["out_o"][qc * 128 : (qc + 1) * 128, :], in_=y)


def build(use_mask=False, for_sim=False):
    key = (use_mask, for_sim)
    if key in _cache:
        return _cache[key]
    nc = bacc.Bacc(
        "TRN2", target_bir_lowering=False, debug=False, num_devices=NCORES
    )
    io = {}
    io["qT"] = nc.dram_tensor("qT", [D, QPC], BF16, kind="ExternalInput").ap()
    io["kT"] = nc.dram_tensor("kT", [D, S], BF16, kind="ExternalInput").ap()
    io["vT"] = nc.dram_tensor("vT", [D, S], BF16, kind="ExternalInput").ap()
    io["qres"] = nc.dram_tensor("qres", [QPC, D], F32, kind="ExternalInput").ap()
    io["wq"] = nc.dram_tensor("wq", [D, HDK], BF16, kind="ExternalInput").ap()
    io["wk"] = nc.dram_tensor("wk", [D, HDK], BF16, kind="ExternalInput").ap()
    io["wfc"] = nc.dram_tensor("wfc", [HDV, D], BF16, kind="ExternalInput").ap()
    for nm in ("bq", "bk"):
        io[nm] = nc.dram_tensor(nm, [HDK], F32, kind="ExternalInput").ap()
    for nm in ("bfc", "gamma", "beta"):
        io[nm] = nc.dram_tensor(nm, [D], F32, kind="ExternalInput").ap()
    if use_mask:
        io["maskb"] = nc.dram_tensor(
            "maskb", [QPC, S], F32, kind="ExternalInput"
        ).ap()
        io["maskbT"] = nc.dram_tensor(
            "maskbT", [S, QPC], F32, kind="ExternalInput"
        ).ap()
    io["attn_o"] = nc.dram_tensor(
        "attn_o", [H, QPC, S], F32, kind="ExternalOutput"
    ).ap()
    io["out_o"] = nc.dram_tensor("out_o", [QPC, D], F32, kind="ExternalOutput").ap()

    from contextlib import ExitStack

    with tile.TileContext(nc) as tc:
        with ExitStack() as ctx:
            _emit(nc, tc, io, use_mask, ctx)
    nc.compile()
    if not for_sim:
        nc.m = get_hw_module(nc.m)
    _cache[key] = nc
    return nc


def make_in_maps(q, k, v, mask, Wq, bq, Wk, bk, Wfc, bfc, gamma, beta, use_mask):
    q = np.asarray(q, np.float32)
    k = np.asarray(k, np.float32)
    v = np.asarray(v, np.float32)
    in_maps = []
    for c in range(NCORES):
        b = c // CORES_PER_B
        r0 = (c % CORES_PER_B) * QPC
        m = {
            "qT": np.ascontiguousarray(q[b, r0 : r0 + QPC, :].T).astype(BF),
            "kT": np.ascontiguousarray(k[b].T).astype(BF),
            "vT": np.ascontiguousarray(v[b].T).astype(BF),
            "qres": np.ascontiguousarray(q[b, r0 : r0 + QPC, :]),
            "wq": np.asarray(Wq, np.float32).astype(BF),
            "wk": np.asarray(Wk, np.float32).astype(BF),
            "wfc": np.asarray(Wfc, np.float32).astype(BF),
            "bq": np.asarray(bq, np.float32),
            "bk": np.asarray(bk, np.float32),
            "bfc": np.asarray(bfc, np.float32),
            "gamma": np.asarray(gamma, np.float32),
            "beta": np.asarray(beta, np.float32),
        }
        if use_mask:
            mb = np.where(np.asarray(mask[b]), np.float32(-1e9), np.float32(0.0))
            m["maskb"] = np.ascontiguousarray(mb[r0 : r0 + QPC, :])
            m["maskbT"] = np.ascontiguousarray(mb[r0 : r0 + QPC, :].T)
        in_maps.append(m)
    return in_maps


def kernel(q, k, v, mask, Wq, bq, Wk, bk, Wfc, bfc, gamma, beta):
    global LAST_EXEC_NS
    use_mask = bool(np.asarray(mask).any())
    nc = build(use_mask=use_mask)
    in_maps = make_in_maps(
        q, k, v, mask, Wq, bq, Wk, bk, Wfc, bfc, gamma, beta, use_mask
    )
    res = run_bass_kernel_spmd(
        nc, in_maps, core_ids=list(range(NCORES)), trace=TRACE
    )
    LAST_EXEC_NS = res.exec_time_ns
    out = np.empty((B, S, D), np.float32)
    attn = np.empty((B, H, S, S), np.float32)
    for c in range(NCORES):
        b = c // CORES_PER_B
        r0 = (c % CORES_PER_B) * QPC
        out[b, r0 : r0 + QPC, :] = res.results[c]["out_o"]
        attn[b, :, r0 : r0 + QPC, :] = res.results[c]["attn_o"]
    return out, attn


# revision 25
# speedup vs baseline: 1.0501x; 1.0501x over previous
"""Trainium2 Bass kernel for nn_MultiHeadAttention_90005334655147.

Math (faithful to the reference):
    qh = (q @ Wq + bq)  -> [B,S,H,DK] -> heads
    kh = (k @ Wk + bk)
    vh = (v @ Wk + bk)          # reference really uses Wk/bk for v
    S_ = (qh @ kh^T) / 8        # per head
    P  = softmax(S_) (mask is all-False in the harness inputs; a masked
                      fallback variant is compiled lazily if needed)
    out_h = P @ vh
    out = concat_h(out_h) @ Wfc + bfc + q   (residual)
    out = LayerNorm(out) * gamma + beta
    returns (out, P)

Sharding: data-parallel over (batch, query-block): core c handles batch
c//4 and query rows [(c%4)*512, (c%4)*512+512). Each core computes kh/vh
for its full batch (duplicated, cheap) -> zero collectives.

Device-side layout strategy (per core):
  - host pre-transposes/casts activations: qT/kT/vT = x.T in bf16 so the
    d-contraction sits on partitions; residual q stays natural fp32.
  - qhT/khT computed as [hdk, rows] (transposed), vh as [rows, hdv]:
    all via natural-layout matmuls.
  - scores are computed TWICE (cheap on PE): once as [q,k] (for the attn
    output + softmax sums) and once as [k,q] (feeds P@V directly).  This
    avoids any on-chip transpose of the big P matrix, whose cost would
    exceed the extra exp pass.
  - exp on ACT with fused accumulate (row sums); normalization on DVE.
  - LayerNorm rstd via exp(-0.5*ln(var+eps)) so ACT stays on the
    natural_log_exp table set (no table switches).
"""

import os
import sys

for _p in ("/opt/trn_rl_repo", "/root/.axon_site/_ro/trn_rl_repo"):
    if os.path.isdir(_p) and _p not in sys.path:
        sys.path.insert(0, _p)

import ml_dtypes
import numpy as np

import concourse.bacc as bacc
import concourse.mybir as mybir
import concourse.tile as tile
from concourse.bass_interp import get_hw_module
from concourse.bass_utils import run_bass_kernel_spmd

B, S, D = 2, 2048, 512
H, DK, DV = 8, 64, 64
HDK = H * DK  # 512
HDV = H * DV  # 512
INV_SCALE = 0.125  # attn / 8.0
LN_EPS = 1e-5

NCORES = 8
CORES_PER_B = NCORES // B  # 4
QPC = S // CORES_PER_B  # 512 query rows per core

F32 = mybir.dt.float32
BF16 = mybir.dt.bfloat16
BF = ml_dtypes.bfloat16

TRACE = bool(int(os.environ.get("KERNEL_TRACE", "0")))
LAST_EXEC_NS = None

if TRACE:
    # Register the axon NTFF profile hook if the image's antenv lacks it.
    try:
        import antenv.axon_hooks as _ah
        from trn_agent_boot.trn_boot import _ntff_profile_via_ctypes

        if _ah.get_axon_ntff_profile_hook() is None:
            _h = _ntff_profile_via_ctypes("/opt/axon/libaxon_pjrt.so")
            if _h is not None:
                _ah.set_axon_ntff_profile_hook(_h)
    except Exception:
        pass

_cache = {}

ADD = mybir.AluOpType.add
SUB = mybir.AluOpType.subtract
MULT = mybir.AluOpType.mult
EXP = mybir.ActivationFunctionType.Exp
LN_ = mybir.ActivationFunctionType.Ln


def _emit(nc, tc, io, use_mask, ctx):
    NQC = QPC // 128  # 4 query-row chunks
    NKC = S // 128  # 16 key-row chunks
    NDC = D // 128  # 4 d chunks
    NMC = HDK // 128  # 4 output-col chunks for projections
    SCT = 1024  # scores psum tile free size (2 banks)
    NSC = S // SCT  # 2 per (h,qc)

    singles = ctx.enter_context(tc.tile_pool(name="singles", bufs=1))
    p32p = ctx.enter_context(tc.tile_pool(name="p32p", bufs=2))
    ptp = ctx.enter_context(tc.tile_pool(name="ptp", bufs=40))
    smallp = ctx.enter_context(tc.tile_pool(name="smallp", bufs=6))
    outp = ctx.enter_context(tc.tile_pool(name="outp", bufs=2))
    ps_sc = ctx.enter_context(tc.tile_pool(name="ps_sc", bufs=2, space="PSUM"))
    ps_sct = ctx.enter_context(tc.tile_pool(name="ps_sct", bufs=3, space="PSUM"))
    ps_pv = ctx.enter_context(tc.tile_pool(name="ps_pv", bufs=1, space="PSUM"))
    if use_mask:
        maskqp = ctx.enter_context(tc.tile_pool(name="maskqp", bufs=2))
        masktp = ctx.enter_context(tc.tile_pool(name="masktp", bufs=2))

    # ---- stage 0: preload constants / inputs ----
    qT_sb = singles.tile([128, NDC, QPC], BF16, tag="qT_sb")
    nc.sync.dma_start(out=qT_sb, in_=io["qT"].rearrange("(c p) q -> p c q", p=128))
    kvp = tc.alloc_tile_pool(name="kvp", bufs=1)
    kT_sb = kvp.tile([128, NDC, S], BF16, tag="kT_sb")
    nc.sync.dma_start(out=kT_sb, in_=io["kT"].rearrange("(c p) q -> p c q", p=128))
    vT_sb = kvp.tile([128, NDC, S], BF16, tag="vT_sb")
    nc.sync.dma_start(out=vT_sb, in_=io["vT"].rearrange("(c p) q -> p c q", p=128))
    qres_sb = singles.tile([128, NQC, D], F32, tag="qres_sb")
    nc.sync.dma_start(out=qres_sb, in_=io["qres"].rearrange("(c p) d -> p c d", p=128))

    wq_sb = singles.tile([128, NDC, HDK], BF16, tag="wq_sb")
    nc.sync.dma_start(out=wq_sb, in_=io["wq"].rearrange("(c p) m -> p c m", p=128))
    wk_sb = singles.tile([128, NDC, HDK], BF16, tag="wk_sb")
    nc.sync.dma_start(out=wk_sb, in_=io["wk"].rearrange("(c p) m -> p c m", p=128))
    wfc_sb = singles.tile([64, H, D], BF16, tag="wfc_sb")
    nc.sync.dma_start(out=wfc_sb, in_=io["wfc"].rearrange("(h p) n -> p h n", p=64))

    bq_sb = singles.tile([128, NMC], F32, tag="bq_sb")
    nc.sync.dma_start(out=bq_sb, in_=io["bq"].rearrange("(c p) -> p c", p=128))
    bk_sb = singles.tile([128, NMC], F32, tag="bk_sb")
    nc.sync.dma_start(out=bk_sb, in_=io["bk"].rearrange("(c p) -> p c", p=128))

    def bcast_row(name):
        t = singles.tile([128, D], F32, tag=name + "B")
        src = io[name].rearrange("(o n) -> o n", o=1).partition_broadcast(128)
        nc.sync.dma_start(out=t, in_=src)
        return t

    bkB = bcast_row("bk")
    bfcB = bcast_row("bfc")
    gammaB = bcast_row("gamma")
    betaB = bcast_row("beta")

    # vh with a ones column appended (row sums of exp come out of the PV
    # matmul for free, in [1, q] orientation)
    vh_sb = singles.tile([128, NKC, H, DV + 1], BF16, tag="vh_sb")
    nc.vector.memset(vh_sb[:, :, :, DV : DV + 1], 1.0)

    # ---- stage 1: projections ----
    qhT_sb = singles.tile([128, NMC, QPC], BF16, tag="qhT_sb")
    for m in range(NMC):
        ps = ps_sct.tile([128, QPC], F32, tag="sct")
        for c in range(NDC):
            nc.tensor.matmul(
                ps,
                lhsT=wq_sb[:, c, m * 128 : (m + 1) * 128],
                rhs=qT_sb[:, c, :],
                start=(c == 0),
                stop=(c == NDC - 1),
            )
        nc.vector.tensor_scalar_add(
            out=qhT_sb[:, m, :], in0=ps, scalar1=bq_sb[:, m : m + 1]
        )

    khT_sb = singles.tile([128, NMC, S], BF16, tag="khT_sb")
    for m in range(NMC):
        for n in range(NSC):
            ps = ps_sc.tile([128, SCT], F32, tag="sc")
            for half in range(SCT // 512):
                for c in range(NDC):
                    nc.tensor.matmul(
                        ps[:, half * 512 : (half + 1) * 512],
                        lhsT=wk_sb[:, c, m * 128 : (m + 1) * 128],
                        rhs=kT_sb[
                            :, c, n * SCT + half * 512 : n * SCT + (half + 1) * 512
                        ],
                        start=(c == 0),
                        stop=(c == NDC - 1),
                    )
            nc.vector.tensor_scalar_add(
                out=khT_sb[:, m, n * SCT : (n + 1) * SCT],
                in0=ps,
                scalar1=bk_sb[:, m : m + 1],
            )

    for kc in range(NKC):
        ps = ps_sct.tile([128, HDV], F32, tag="sct")
        for c in range(NDC):
            nc.tensor.matmul(
                ps,
                lhsT=vT_sb[:, c, kc * 128 : (kc + 1) * 128],
                rhs=wk_sb[:, c, :],
                start=(c == 0),
                stop=(c == NDC - 1),
            )
        nc.vector.tensor_tensor(
            out=vh_sb[:, kc, :, 0:DV],
            in0=ps.rearrange("p (h d) -> p h d", h=H),
            in1=bkB.rearrange("p (h d) -> p h d", h=H),
            op=ADD,
        )

    kvp.release()

    # ---- stage 2: attention per head ----
    outTn_sb = singles.tile([64, H, QPC], BF16, tag="outTn_sb")
    ones64 = singles.tile([65, 64], F32, tag="ones64")
    nc.vector.memset(ones64, 1.0)

    # Heads are processed in pairs: the even head's dk rows live on
    # partitions 0-63, the odd head's on 64-127, so their K=64 matmuls
    # issue to distinct PE row-groups (tile_position auto-derived from
    # base_partition) and run concurrently in the array.
    for pr in range(H // 2):
        heads = (2 * pr, 2 * pr + 1)
        pts_all = {h: [] for h in heads}
        for kc in range(NKC):
            # scores^T [k,q] for both heads (packed pair)
            pss = {}
            for h in heads:
                hp = (h % 2) * 64
                ps = ps_sct.tile([128, QPC], F32, tag="sct", name=f"sct{h}_{kc}")
                nc.tensor.matmul(
                    ps,
                    lhsT=khT_sb[hp : hp + 64, pr, kc * 128 : (kc + 1) * 128],
                    rhs=qhT_sb[hp : hp + 64, pr, :],
                    start=True,
                    stop=True,
                )
                pss[h] = ps
            if use_mask:
                mt = masktp.tile([128, QPC], F32, tag="mt")
                nc.sync.dma_start(
                    out=mt, in_=io["maskbT"][kc * 128 : (kc + 1) * 128, :]
                )
                for h in heads:
                    nc.vector.tensor_tensor(out=pss[h], in0=pss[h], in1=mt, op=ADD)
            for h in heads:
                pt = ptp.tile([128, QPC], BF16, tag="pt", name=f"pt{h}_{kc}")
                nc.scalar.activation(out=pt, in_=pss[h], func=EXP, scale=INV_SCALE)
                pts_all[h].append(pt)

            # every 4th k-chunk, interleave one [q,k] scores burst so the
            # PE/ACT streams stay dense
            if kc % 4 == 3:
                qc = kc // 4
                p32s = {h: p32p.tile([128, S], F32, tag="p32", name=f"p32_{h}") for h in heads}
                sums = {
                    h: smallp.tile([128, NSC + 2], F32, tag="sums2", name=f"sums{h}") for h in heads
                }
                for n in range(NSC):
                    pp = {h: ps_sc.tile([128, SCT], F32, tag="sc", name=f"sc{h}_{n}") for h in heads}
                    for half in range(SCT // 512):
                        for h in heads:
                            hp = (h % 2) * 64
                            o = n * SCT + half * 512
                            nc.tensor.matmul(
                                pp[h][:, half * 512 : (half + 1) * 512],
                                lhsT=qhT_sb[
                                    hp : hp + 64, pr, qc * 128 : (qc + 1) * 128
                                ],
                                rhs=khT_sb[hp : hp + 64, pr, o : o + 512],
                                start=True,
                                stop=True,
                            )
                    if use_mask:
                        mq = maskqp.tile([128, SCT], F32, tag="mq")
                        nc.sync.dma_start(
                            out=mq,
                            in_=io["maskb"][
                                qc * 128 : (qc + 1) * 128, n * SCT : (n + 1) * SCT
                            ],
                        )
                        for h in heads:
                            nc.vector.tensor_tensor(
                                out=pp[h], in0=pp[h], in1=mq, op=ADD
                            )
                    for h in heads:
                        nc.scalar.activation(
                            out=p32s[h][:, n * SCT : (n + 1) * SCT],
                            in_=pp[h],
                            func=EXP,
                            scale=INV_SCALE,
                            accum_out=sums[h][:, n : n + 1],
                        )
                for h in heads:
                    s2 = sums[h]
                    nc.vector.tensor_tensor(
                        out=s2[:, NSC : NSC + 1],
                        in0=s2[:, 0:1],
                        in1=s2[:, 1:2],
                        op=ADD,
                    )
                    nc.vector.reciprocal(
                        out=s2[:, NSC + 1 : NSC + 2], in_=s2[:, NSC : NSC + 1]
                    )
                    nc.vector.tensor_scalar_mul(
                        out=p32s[h], in0=p32s[h], scalar1=s2[:, NSC + 1 : NSC + 2]
                    )
                    nc.sync.dma_start(
                        out=io["attn_o"][h, qc * 128 : (qc + 1) * 128, :],
                        in_=p32s[h],
                    )

        # per-head PV accumulation as one unbroken 16-matmul burst (keeps
        # the PE stream dense), then normalize by the (transposed) softmax
        # sums: reciprocal of the ones-column row, broadcast across
        # partitions via a K=1 outer-product matmul
        for h in heads:
            pv = ps_pv.tile([DV + 1, QPC], F32, tag="pv", name=f"pv{h}")
            for kc in range(NKC):
                nc.tensor.matmul(
                    pv,
                    lhsT=vh_sb[:, kc, h, :],
                    rhs=pts_all[h][kc],
                    start=(kc == 0),
                    stop=(kc == NKC - 1),
                )
            rt = smallp.tile([65, QPC], F32, tag="rt")
            nc.vector.reciprocal(out=rt[64:65, :], in_=pv[DV : DV + 1, :])
            rtp = ps_sc.tile([64, QPC], F32, tag="sc")
            nc.tensor.matmul(
                rtp, lhsT=ones64[64:65, :], rhs=rt[64:65, :], start=True, stop=True
            )
            rtb = smallp.tile([64, QPC], F32, tag="rtb")
            nc.vector.tensor_copy(out=rtb, in_=rtp)
            nc.vector.tensor_tensor(
                out=outTn_sb[:, h, :], in0=pv[0:DV, :], in1=rtb, op=MULT
            )

    # ---- stage 3: fc + residual + layernorm ----
    # All Ln/Exp calls batched at the end so the ACT table set is switched
    # at most twice instead of per-qc.
    eps_sb = singles.tile([128, 1], F32, tag="eps_sb")
    nc.vector.memset(eps_sb, LN_EPS)
    mvs = singles.tile([128, NQC, 2], F32, tag="mvs")
    lns = singles.tile([128, NQC], F32, tag="lns")
    rstds = singles.tile([128, NQC], F32, tag="rstds")
    xs = []
    for qc in range(NQC):
        fc = ps_sc.tile([128, D], F32, tag="sc")
        for h in range(H):
            nc.tensor.matmul(
                fc,
                lhsT=outTn_sb[:, h, qc * 128 : (qc + 1) * 128],
                rhs=wfc_sb[:, h, :],
                start=(h == 0),
                stop=(h == H - 1),
            )
        x = outp.tile([128, D], F32, tag="x", bufs=NQC)
        nc.vector.tensor_tensor(out=x, in0=fc, in1=qres_sb[:, qc, :], op=ADD)
        nc.vector.tensor_tensor(out=x, in0=x, in1=bfcB, op=ADD)
        st = smallp.tile([128, 6], F32, tag="st")
        nc.vector.bn_stats(out=st, in_=x)
        nc.vector.bn_aggr(out=mvs[:, qc, :], in_=st)
        xs.append(x)
    nc.scalar.activation(out=lns, in_=mvs[:, :, 1], func=LN_, bias=eps_sb, scale=1.0)
    nc.scalar.activation(out=rstds, in_=lns, func=EXP, scale=-0.5)
    for qc in range(NQC):
        y = outp.tile([128, D], F32, tag="y")
        nc.vector.tensor_scalar(
            out=y,
            in0=xs[qc],
            scalar1=mvs[:, qc, 0:1],
            scalar2=rstds[:, qc : qc + 1],
            op0=SUB,
            op1=MULT,
        )
        nc.vector.tensor_tensor(out=y, in0=y, in1=gammaB, op=MULT)
        nc.vector.tensor_tensor(out=y, in0=y, in1=betaB, op=ADD)
        nc.sync.dma_start(out=io["out_o"][qc * 128 : (qc + 1) * 128, :], in_=y)


def build(use_mask=False, for_sim=False):
    key = (use_mask, for_sim)
    if key in _cache:
        return _cache[key]
    nc = bacc.Bacc(
        "TRN2", target_bir_lowering=False, debug=False, num_devices=NCORES
    )
    io = {}
    io["qT"] = nc.dram_tensor("qT", [D, QPC], BF16, kind="ExternalInput").ap()
    io["kT"] = nc.dram_tensor("kT", [D, S], BF16, kind="ExternalInput").ap()
    io["vT"] = nc.dram_tensor("vT", [D, S], BF16, kind="ExternalInput").ap()
    io["qres"] = nc.dram_tensor("qres", [QPC, D], F32, kind="ExternalInput").ap()
    io["wq"] = nc.dram_tensor("wq", [D, HDK], BF16, kind="ExternalInput").ap()
    io["wk"] = nc.dram_tensor("wk", [D, HDK], BF16, kind="ExternalInput").ap()
    io["wfc"] = nc.dram_tensor("wfc", [HDV, D], BF16, kind="ExternalInput").ap()
    for nm in ("bq", "bk"):
        io[nm] = nc.dram_tensor(nm, [HDK], F32, kind="ExternalInput").ap()
    for nm in ("bfc", "gamma", "beta"):
        io[nm] = nc.dram_tensor(nm, [D], F32, kind="ExternalInput").ap()
    if use_mask:
        io["maskb"] = nc.dram_tensor(
            "maskb", [QPC, S], F32, kind="ExternalInput"
        ).ap()
        io["maskbT"] = nc.dram_tensor(
            "maskbT", [S, QPC], F32, kind="ExternalInput"
        ).ap()
    io["attn_o"] = nc.dram_tensor(
        "attn_o", [H, QPC, S], F32, kind="ExternalOutput"
    ).ap()
    io["out_o"] = nc.dram_tensor("out_o", [QPC, D], F32, kind="ExternalOutput").ap()

    from contextlib import ExitStack

    with tile.TileContext(nc) as tc:
        with ExitStack() as ctx:
            _emit(nc, tc, io, use_mask, ctx)
    nc.compile()
    if not for_sim:
        nc.m = get_hw_module(nc.m)
    _cache[key] = nc
    return nc


def make_in_maps(q, k, v, mask, Wq, bq, Wk, bk, Wfc, bfc, gamma, beta, use_mask):
    q = np.asarray(q, np.float32)
    k = np.asarray(k, np.float32)
    v = np.asarray(v, np.float32)
    in_maps = []
    for c in range(NCORES):
        b = c // CORES_PER_B
        r0 = (c % CORES_PER_B) * QPC
        m = {
            "qT": np.ascontiguousarray(q[b, r0 : r0 + QPC, :].T).astype(BF),
            "kT": np.ascontiguousarray(k[b].T).astype(BF),
            "vT": np.ascontiguousarray(v[b].T).astype(BF),
            "qres": np.ascontiguousarray(q[b, r0 : r0 + QPC, :]),
            "wq": np.asarray(Wq, np.float32).astype(BF),
            "wk": np.asarray(Wk, np.float32).astype(BF),
            "wfc": np.asarray(Wfc, np.float32).astype(BF),
            "bq": np.asarray(bq, np.float32),
            "bk": np.asarray(bk, np.float32),
            "bfc": np.asarray(bfc, np.float32),
            "gamma": np.asarray(gamma, np.float32),
            "beta": np.asarray(beta, np.float32),
        }
        if use_mask:
            mb = np.where(np.asarray(mask[b]), np.float32(-1e9), np.float32(0.0))
            m["maskb"] = np.ascontiguousarray(mb[r0 : r0 + QPC, :])
            m["maskbT"] = np.ascontiguousarray(mb[r0 : r0 + QPC, :].T)
        in_maps.append(m)
    return in_maps


def kernel(q, k, v, mask, Wq, bq, Wk, bk, Wfc, bfc, gamma, beta):
    global LAST_EXEC_NS
    use_mask = bool(np.asarray(mask).any())
    nc = build(use_mask=use_mask)
    in_maps = make_in_maps(
        q, k, v, mask, Wq, bq, Wk, bk, Wfc, bfc, gamma, beta, use_mask
    )
    res = run_bass_kernel_spmd(
        nc, in_maps, core_ids=list(range(NCORES)), trace=TRACE
    )
    LAST_EXEC_NS = res.exec_time_ns
    out = np.empty((B, S, D), np.float32)
    attn = np.empty((B, H, S, S), np.float32)
    for c in range(NCORES):
        b = c // CORES_PER_B
        r0 = (c % CORES_PER_B) * QPC
        out[b, r0 : r0 + QPC, :] = res.results[c]["out_o"]
        attn[b, :, r0 : r0 + QPC, :] = res.results[c]["attn_o"]
    return out, attn
